# revision 8
# baseline (speedup 1.0000x reference)
"""Bilinear causal attention (nn_Attention_34772055228779) on 8 trn2 cores.

reference:
  scores[i,k] = x[i] @ W_bi[k] @ x[i]          [512, 512]
  attn = softmax(scores + causal_mask, axis=1)
  out  = (attn @ x) @ W_out.T                  [512, 512]

Device strategy (tensor-parallel over score columns, per sharding hint):
  core m holds W_bi[64m:64(m+1)]  (64 MiB fp32)
  stage A: for each local k: Y_k = X @ W_k  (fp32r matmuls, lhsT = X^T resident)
           scores[:, k] = rowsum(Y_k * X)   (fused DVE scalar_tensor_tensor)
  AllToAll over the [8 x 64-row, 64-col] score shard blocks: core m ends up
           with rows [64m, 64m+64) of the FULL score matrix.
  tail:    masked softmax rows (ACT exp with fused accum), A^T via PE
           transpose, O^T = X^T A^T, Y = O @ W_out^T, DMA 64 output rows.
  host:    concatenates the 8 row blocks.
"""
import numpy as np

N_CTX = 512
D = 512
NCORES = 8
KSH = N_CTX // NCORES      # 64 score columns per core
RSH = N_CTX // NCORES      # 64 output rows per core
NEG_INF = -1e30

_nc_cache = None


def _build(timing_loop=0, use_collective=True, num_devices=NCORES):
    """Build the Bass module.

    timing_loop=R>0 wraps the whole per-core body in a hardware For_i loop
    (R iterations) for slope timing; collectives can't sit in control flow,
    so timing variants pass use_collective=False (the gather DMA then reads
    the pre-collective buffer -- wrong data, identical shapes/costs).
    """
    import concourse.mybir as mybir
    import concourse.tile as tile
    from concourse import bacc

    f32 = mybir.dt.float32
    f32r = mybir.dt.float32r
    Alu = mybir.AluOpType
    Act = mybir.ActivationFunctionType

    nc = bacc.Bacc(
        "TRN2", target_bir_lowering=False, debug=False,
        enable_asserts=False, num_devices=num_devices,
    )

    x_t = nc.dram_tensor("x", [N_CTX, D], f32, kind="ExternalInput").ap()
    xt_t = nc.dram_tensor("xt", [D, N_CTX], f32, kind="ExternalInput").ap()
    wbi_t = nc.dram_tensor("wbi", [KSH, D, D], f32, kind="ExternalInput").ap()
    woutt_t = nc.dram_tensor("wout_t", [D, D], f32, kind="ExternalInput").ap()
    mask_t = nc.dram_tensor("mask", [RSH, N_CTX], f32, kind="ExternalInput").ap()
    ident_t = nc.dram_tensor("ident", [128, 128], f32, kind="ExternalInput").ap()
    out_t = nc.dram_tensor("out", [RSH, D], f32, kind="ExternalOutput").ap()

    with tile.TileContext(nc) as tc:
        with (
            tc.tile_pool(name="const", bufs=1) as cpool,
            tc.tile_pool(name="wstream", bufs=4) as wpool,
            tc.tile_pool(name="scratch", bufs=3) as spool,
            tc.tile_pool(name="small", bufs=1) as mpool,
            tc.tile_pool(name="psA", bufs=6, space="PSUM") as ppA,
            tc.tile_pool(name="psB", bufs=2, space="PSUM") as ppB,
            tc.tile_pool(name="dram", bufs=1, space="DRAM") as dpool,
        ):
            # ---- resident loads (outside any timing loop) -----------------
            x_sb, xt_sb, woutt_sb = [], [], []
            for t in range(4):
                a = cpool.tile([128, N_CTX], f32, tag=f"x{t}", name=f"x{t}")
                nc.sync.dma_start(a[:], x_t[t * 128:(t + 1) * 128, :])
                x_sb.append(a)
                b = cpool.tile([128, N_CTX], f32r, tag=f"xt{t}", name=f"xt{t}")
                nc.sync.dma_start(b[:], xt_t[t * 128:(t + 1) * 128, :].bitcast(f32r))
                xt_sb.append(b)
                c = cpool.tile([128, D], f32, tag=f"wo{t}", name=f"wo{t}")
                woutt_sb.append(c)
            mask_sb = cpool.tile([RSH, N_CTX], f32, tag="mask")
            ident_sb = cpool.tile([128, 128], f32, tag="ident")
            scores_sb = [
                cpool.tile([128, KSH], f32, tag=f"sc{t}", name=f"sc{t}")
                for t in range(4)
            ]
            agin = dpool.tile([N_CTX, KSH], f32, tag="agin")
            agout = dpool.tile([N_CTX, KSH], f32, tag="agout")

            def body():
                # ---- stage A: local score columns -------------------------
                for kk in range(KSH):
                    wk = wpool.tile([128, 4, D], f32r, tag="wk", name="wk")
                    nc.sync.dma_start(
                        wk[:],
                        wbi_t[kk].rearrange("(dt p) e -> p dt e", p=128).bitcast(f32r),
                    )
                    for nt in range(4):
                        yp = ppA.tile([128, D], f32, tag="yp", name="yp")
                        for dt in range(4):
                            nc.tensor.matmul(
                                yp[:],
                                lhsT=xt_sb[dt][:, nt * 128:(nt + 1) * 128],
                                rhs=wk[:, dt, :],
                                start=(dt == 0),
                                stop=(dt == 3),
                            )
                        scr = spool.tile([128, D], f32, tag="stt_out", name="scr")
                        nc.vector.scalar_tensor_tensor(
                            out=scr[:], in0=yp[:], scalar=1.0, in1=x_sb[nt][:],
                            op0=Alu.mult, op1=Alu.mult,
                            accum_out=scores_sb[nt][:, kk:kk + 1],
                        )

                # tail-only constants: emitted after stage A so their DMAs
                # don't delay the first W_k prefetches
                nc.sync.dma_start(mask_sb[:], mask_t[:])
                nc.sync.dma_start(ident_sb[:], ident_t[:])
                for t in range(4):
                    nc.sync.dma_start(
                        woutt_sb[t][:], woutt_t[t * 128:(t + 1) * 128, :])

                # ---- AllToAll: shard columns -> shard rows ----------------
                for nt in range(4):
                    nc.sync.dma_start(
                        agin[nt * 128:(nt + 1) * 128, :], scores_sb[nt][:])
                if use_collective:
                    nc.gpsimd.collective_compute(
                        "AllToAll",
                        mybir.AluOpType.bypass,
                        replica_groups=[list(range(NCORES))],
                        ins=[agin[:].opt()],
                        outs=[agout[:].opt()],
                    )
                    coll_out = agout
                else:
                    coll_out = agin
                # rows of the full score matrix for this core: [64, 512]
                sfull = mpool.tile([RSH, N_CTX], f32, tag="sfull", name="sfull")
                nc.sync.dma_start(
                    sfull[:].rearrange("i (r k) -> i r k", r=NCORES),
                    coll_out[:].rearrange("(r i) k -> i r k", r=NCORES),
                )

                # ---- masked softmax over the 64 rows ----------------------
                sm = mpool.tile([RSH, N_CTX], f32, tag="sm", name="sm")
                nc.vector.tensor_tensor(
                    out=sm[:], in0=sfull[:], in1=mask_sb[:], op=Alu.add)
                negm = mpool.tile([RSH, 1], f32, tag="negm", name="negm")
                nc.vector.reduce_max(negm[:], sm[:], axis=mybir.AxisListType.X,
                                     negate=True)
                esb = mpool.tile([RSH, N_CTX], f32, tag="esb", name="esb")
                den = mpool.tile([RSH, 1], f32, tag="den", name="den")
                nc.scalar.activation(
                    esb[:], sm[:], Act.Exp, bias=negm[:], scale=1.0,
                    accum_out=den[:])
                rden = mpool.tile([RSH, 1], f32, tag="rden", name="rden")
                nc.vector.reciprocal(rden[:], den[:])
                a_sb = mpool.tile([RSH, N_CTX], f32, tag="a_sb", name="a_sb")
                nc.vector.tensor_scalar_mul(a_sb[:], esb[:], rden[:])

                # ---- A^T via PE transpose: [64, 512] -> 4x [128, 64] ------
                at_sb = []
                for kt in range(4):
                    tp = ppB.tile([128, 512], f32, tag="tail", name="tp")
                    nc.tensor.transpose(
                        tp[:, 0:RSH],
                        a_sb[:, kt * 128:(kt + 1) * 128],
                        ident_sb[0:RSH, 0:RSH],
                    )
                    at = mpool.tile([128, RSH], f32, tag=f"at{kt}", name=f"at{kt}")
                    nc.scalar.copy(at[:], tp[:, 0:RSH])
                    at_sb.append(at)

                # ---- O^T = X^T @ A^T : [512(e), 64(i)] --------------------
                ot_sb = []
                for et in range(4):
                    op = ppB.tile([128, 512], f32, tag="tail", name="op")
                    for kt in range(4):
                        nc.tensor.matmul(
                            op[:, 0:RSH],
                            lhsT=x_sb[kt][:, et * 128:(et + 1) * 128],
                            rhs=at_sb[kt][:],
                            start=(kt == 0),
                            stop=(kt == 3),
                        )
                    ot = mpool.tile([128, RSH], f32, tag=f"ot{et}", name=f"ot{et}")
                    nc.scalar.copy(ot[:], op[:, 0:RSH])
                    ot_sb.append(ot)

                # ---- Y = O @ W_out^T : [64(i), 512(f)] --------------------
                ypz = ppB.tile([128, 512], f32, tag="tail", name="ypz")
                for et in range(4):
                    nc.tensor.matmul(
                        ypz[0:RSH, :],
                        lhsT=ot_sb[et][:],
                        rhs=woutt_sb[et][:],
                        start=(et == 0),
                        stop=(et == 3),
                    )
                y_sb = mpool.tile([RSH, D], f32, tag="y_sb", name="y_sb")
                nc.scalar.copy(y_sb[:], ypz[0:RSH, :])
                nc.sync.dma_start(out_t[:], y_sb[:])

            if timing_loop:
                with tc.For_i(0, timing_loop, 1):
                    body()
            else:
                body()

    nc.compile()
    return nc


def _make_in_maps(x, W_bi, W_out):
    x = np.ascontiguousarray(np.asarray(x, dtype=np.float32))
    W_bi = np.asarray(W_bi, dtype=np.float32)
    W_out = np.asarray(W_out, dtype=np.float32)
    xt = np.ascontiguousarray(x.T)
    woutt = np.ascontiguousarray(W_out.T)
    mask_full = np.triu(np.full((N_CTX, N_CTX), NEG_INF, dtype=np.float32), 1)
    ident = np.eye(128, dtype=np.float32)
    in_maps = []
    for m in range(NCORES):
        in_maps.append({
            "x": x,
            "xt": xt,
            "wbi": np.ascontiguousarray(W_bi[m * KSH:(m + 1) * KSH]),
            "wout_t": woutt,
            "mask": np.ascontiguousarray(mask_full[m * RSH:(m + 1) * RSH]),
            "ident": ident,
        })
    return in_maps


def kernel(x, W_bi, W_out):
    global _nc_cache
    import time as _time
    from concourse.bass_utils import run_bass_kernel_spmd

    if _nc_cache is None:
        _nc_cache = _build()
    nc = _nc_cache
    in_maps = _make_in_maps(x, W_bi, W_out)
    last_exc = None
    for attempt in range(3):
        try:
            res = run_bass_kernel_spmd(nc, in_maps, core_ids=list(range(NCORES)),
                                       trace=False)
            break
        except Exception as e:  # transient NRT/axon wedges recover on retry
            last_exc = e
            _time.sleep(5.0 * (attempt + 1))
    else:
        raise last_exc
    out = np.concatenate([res.results[m]["out"] for m in range(NCORES)], axis=0)
    return np.ascontiguousarray(out, dtype=np.float32)


# revision 11
# speedup vs baseline: 1.1169x; 1.1169x over previous
"""Bilinear causal attention (nn_Attention_34772055228779) on 8 trn2 cores.

reference:
  scores[i,k] = x[i] @ W_bi[k] @ x[i]          [512, 512]
  attn = softmax(scores + causal_mask, axis=1)
  out  = (attn @ x) @ W_out.T                  [512, 512]

Device strategy (tensor-parallel over score columns, per sharding hint):
  core m holds W_bi[64m:64(m+1)]  (64 MiB fp32)
  stage A: for each local k: Y_k = X @ W_k  (fp32r matmuls, lhsT = X^T resident)
           scores[:, k] = rowsum(Y_k * X)   (fused DVE scalar_tensor_tensor)
  AllToAll over the [8 x 64-row, 64-col] score shard blocks: core m ends up
           with rows [64m, 64m+64) of the FULL score matrix.
  tail:    masked softmax rows (ACT exp with fused accum), A^T via PE
           transpose, O^T = X^T A^T, Y = O @ W_out^T, DMA 64 output rows.
  host:    concatenates the 8 row blocks.
"""
import numpy as np

N_CTX = 512
D = 512
NCORES = 8
KSH = N_CTX // NCORES      # 64 score columns per core
RSH = N_CTX // NCORES      # 64 output rows per core
NEG_INF = -1e30

_nc_cache = None


def _build(timing_loop=0, use_collective=True, num_devices=NCORES,
           stage_a="base", wbufs=4):
    """Build the Bass module.

    timing_loop=R>0 wraps the whole per-core body in a hardware For_i loop
    (R iterations) for slope timing; collectives can't sit in control flow,
    so timing variants pass use_collective=False (the gather DMA then reads
    the pre-collective buffer -- wrong data, identical shapes/costs).
    """
    import concourse.mybir as mybir
    import concourse.tile as tile
    from concourse import bacc

    f32 = mybir.dt.float32
    f32r = mybir.dt.float32r
    Alu = mybir.AluOpType
    Act = mybir.ActivationFunctionType

    nc = bacc.Bacc(
        "TRN2", target_bir_lowering=False, debug=False,
        enable_asserts=False, num_devices=num_devices,
    )

    x_t = nc.dram_tensor("x", [N_CTX, D], f32, kind="ExternalInput").ap()
    xt_t = nc.dram_tensor("xt", [D, N_CTX], f32, kind="ExternalInput").ap()
    wbi_t = nc.dram_tensor("wbi", [KSH, D, D], f32, kind="ExternalInput").ap()
    woutt_t = nc.dram_tensor("wout_t", [D, D], f32, kind="ExternalInput").ap()
    mask_t = nc.dram_tensor("mask", [RSH, N_CTX], f32, kind="ExternalInput").ap()
    ident_t = nc.dram_tensor("ident", [128, 128], f32, kind="ExternalInput").ap()
    out_t = nc.dram_tensor("out", [RSH, D], f32, kind="ExternalOutput").ap()

    with tile.TileContext(nc) as tc:
        with (
            tc.tile_pool(name="const", bufs=1) as cpool,
            tc.tile_pool(name="wstream", bufs=wbufs) as wpool,
            tc.tile_pool(name="scratch", bufs=3) as spool,
            tc.tile_pool(name="small", bufs=1) as mpool,
            tc.tile_pool(name="psA", bufs=6, space="PSUM") as ppA,
            tc.tile_pool(name="psB", bufs=2, space="PSUM") as ppB,
            tc.tile_pool(name="dram", bufs=1, space="DRAM") as dpool,
        ):
            # ---- resident loads (outside any timing loop) -----------------
            x_sb, xt_sb, woutt_sb = [], [], []
            for t in range(4):
                a = cpool.tile([128, N_CTX], f32, tag=f"x{t}", name=f"x{t}")
                nc.sync.dma_start(a[:], x_t[t * 128:(t + 1) * 128, :])
                x_sb.append(a)
                b = cpool.tile([128, N_CTX], f32r, tag=f"xt{t}", name=f"xt{t}")
                nc.sync.dma_start(b[:], xt_t[t * 128:(t + 1) * 128, :].bitcast(f32r))
                xt_sb.append(b)
                c = cpool.tile([128, D], f32, tag=f"wo{t}", name=f"wo{t}")
                woutt_sb.append(c)
            mask_sb = cpool.tile([RSH, N_CTX], f32, tag="mask")
            ident_sb = cpool.tile([128, 128], f32, tag="ident")
            scores_sb = [
                cpool.tile([128, KSH], f32, tag=f"sc{t}", name=f"sc{t}")
                for t in range(4)
            ]
            agin = dpool.tile([N_CTX, KSH], f32, tag="agin")
            agout = dpool.tile([N_CTX, KSH], f32, tag="agout")

            def load_wk(kk):
                wk = wpool.tile([128, 4, D], f32r, tag="wk", name="wk")
                nc.sync.dma_start(
                    wk[:],
                    wbi_t[kk].rearrange("(dt p) e -> p dt e", p=128).bitcast(f32r),
                )
                return wk

            def emit_stt(yp, nt, kk):
                scr = spool.tile([128, D], f32, tag="stt_out", name="scr")
                nc.vector.scalar_tensor_tensor(
                    out=scr[:], in0=yp[:], scalar=1.0, in1=x_sb[nt][:],
                    op0=Alu.mult, op1=Alu.mult,
                    accum_out=scores_sb[nt][:, kk:kk + 1],
                )

            def stage_a_base():
                for kk in range(KSH):
                    wk = load_wk(kk)
                    for nt in range(4):
                        yp = ppA.tile([128, D], f32, tag="yp", name="yp")
                        for dt in range(4):
                            nc.tensor.matmul(
                                yp[:],
                                lhsT=xt_sb[dt][:, nt * 128:(nt + 1) * 128],
                                rhs=wk[:, dt, :],
                                start=(dt == 0),
                                stop=(dt == 3),
                            )
                        emit_stt(yp, nt, kk)

            def stage_a_kpair():
                # process k in pairs; consecutive matmuls share the same
                # stationary lhsT tile (halves PE weight reloads)
                for kk in range(0, KSH, 2):
                    wk0 = load_wk(kk)
                    wk1 = load_wk(kk + 1)
                    for nt in range(4):
                        yp0 = ppA.tile([128, D], f32, tag="yp", name="yp")
                        yp1 = ppA.tile([128, D], f32, tag="yp", name="yp")
                        for dt in range(4):
                            lhsT = xt_sb[dt][:, nt * 128:(nt + 1) * 128]
                            nc.tensor.matmul(
                                yp0[:], lhsT=lhsT, rhs=wk0[:, dt, :],
                                start=(dt == 0), stop=(dt == 3),
                                skip_group_check=True)
                            nc.tensor.matmul(
                                yp1[:], lhsT=lhsT, rhs=wk1[:, dt, :],
                                start=(dt == 0), stop=(dt == 3),
                                skip_group_check=True)
                        emit_stt(yp0, nt, kk)
                        emit_stt(yp1, nt, kk + 1)

            def body():
                # ---- stage A: local score columns -------------------------
                if stage_a == "kpair":
                    stage_a_kpair()
                else:
                    stage_a_base()

                # tail-only constants: emitted after stage A so their DMAs
                # don't delay the first W_k prefetches
                nc.sync.dma_start(mask_sb[:], mask_t[:])
                nc.sync.dma_start(ident_sb[:], ident_t[:])
                for t in range(4):
                    nc.sync.dma_start(
                        woutt_sb[t][:], woutt_t[t * 128:(t + 1) * 128, :])

                # ---- AllToAll: shard columns -> shard rows ----------------
                for nt in range(4):
                    nc.sync.dma_start(
                        agin[nt * 128:(nt + 1) * 128, :], scores_sb[nt][:])
                if use_collective:
                    nc.gpsimd.collective_compute(
                        "AllToAll",
                        mybir.AluOpType.bypass,
                        replica_groups=[list(range(NCORES))],
                        ins=[agin[:].opt()],
                        outs=[agout[:].opt()],
                    )
                    coll_out = agout
                else:
                    coll_out = agin
                # rows of the full score matrix for this core: [64, 512]
                sfull = mpool.tile([RSH, N_CTX], f32, tag="sfull", name="sfull")
                nc.sync.dma_start(
                    sfull[:].rearrange("i (r k) -> i r k", r=NCORES),
                    coll_out[:].rearrange("(r i) k -> i r k", r=NCORES),
                )

                # ---- masked softmax over the 64 rows ----------------------
                sm = mpool.tile([RSH, N_CTX], f32, tag="sm", name="sm")
                nc.vector.tensor_tensor(
                    out=sm[:], in0=sfull[:], in1=mask_sb[:], op=Alu.add)
                negm = mpool.tile([RSH, 1], f32, tag="negm", name="negm")
                nc.vector.reduce_max(negm[:], sm[:], axis=mybir.AxisListType.X,
                                     negate=True)
                esb = mpool.tile([RSH, N_CTX], f32, tag="esb", name="esb")
                den = mpool.tile([RSH, 1], f32, tag="den", name="den")
                nc.scalar.activation(
                    esb[:], sm[:], Act.Exp, bias=negm[:], scale=1.0,
                    accum_out=den[:])
                rden = mpool.tile([RSH, 1], f32, tag="rden", name="rden")
                nc.vector.reciprocal(rden[:], den[:])
                a_sb = mpool.tile([RSH, N_CTX], f32, tag="a_sb", name="a_sb")
                nc.vector.tensor_scalar_mul(a_sb[:], esb[:], rden[:])

                # ---- A^T via PE transpose: [64, 512] -> 4x [128, 64] ------
                at_sb = []
                for kt in range(4):
                    tp = ppB.tile([128, 512], f32, tag="tail", name="tp")
                    nc.tensor.transpose(
                        tp[:, 0:RSH],
                        a_sb[:, kt * 128:(kt + 1) * 128],
                        ident_sb[0:RSH, 0:RSH],
                    )
                    at = mpool.tile([128, RSH], f32, tag=f"at{kt}", name=f"at{kt}")
                    nc.scalar.copy(at[:], tp[:, 0:RSH])
                    at_sb.append(at)

                # ---- O^T = X^T @ A^T : [512(e), 64(i)] --------------------
                ot_sb = []
                for et in range(4):
                    op = ppB.tile([128, 512], f32, tag="tail", name="op")
                    for kt in range(4):
                        nc.tensor.matmul(
                            op[:, 0:RSH],
                            lhsT=x_sb[kt][:, et * 128:(et + 1) * 128],
                            rhs=at_sb[kt][:],
                            start=(kt == 0),
                            stop=(kt == 3),
                        )
                    ot = mpool.tile([128, RSH], f32, tag=f"ot{et}", name=f"ot{et}")
                    nc.scalar.copy(ot[:], op[:, 0:RSH])
                    ot_sb.append(ot)

                # ---- Y = O @ W_out^T : [64(i), 512(f)] --------------------
                ypz = ppB.tile([128, 512], f32, tag="tail", name="ypz")
                for et in range(4):
                    nc.tensor.matmul(
                        ypz[0:RSH, :],
                        lhsT=ot_sb[et][:],
                        rhs=woutt_sb[et][:],
                        start=(et == 0),
                        stop=(et == 3),
                    )
                y_sb = mpool.tile([RSH, D], f32, tag="y_sb", name="y_sb")
                nc.scalar.copy(y_sb[:], ypz[0:RSH, :])
                nc.sync.dma_start(out_t[:], y_sb[:])

            if timing_loop:
                with tc.For_i(0, timing_loop, 1):
                    body()
            else:
                body()

    nc.compile()
    return nc


def _make_in_maps(x, W_bi, W_out):
    x = np.ascontiguousarray(np.asarray(x, dtype=np.float32))
    W_bi = np.asarray(W_bi, dtype=np.float32)
    W_out = np.asarray(W_out, dtype=np.float32)
    xt = np.ascontiguousarray(x.T)
    woutt = np.ascontiguousarray(W_out.T)
    mask_full = np.triu(np.full((N_CTX, N_CTX), NEG_INF, dtype=np.float32), 1)
    ident = np.eye(128, dtype=np.float32)
    in_maps = []
    for m in range(NCORES):
        in_maps.append({
            "x": x,
            "xt": xt,
            "wbi": np.ascontiguousarray(W_bi[m * KSH:(m + 1) * KSH]),
            "wout_t": woutt,
            "mask": np.ascontiguousarray(mask_full[m * RSH:(m + 1) * RSH]),
            "ident": ident,
        })
    return in_maps


def kernel(x, W_bi, W_out):
    global _nc_cache
    import time as _time
    from concourse.bass_utils import run_bass_kernel_spmd

    if _nc_cache is None:
        _nc_cache = _build()
    nc = _nc_cache
    in_maps = _make_in_maps(x, W_bi, W_out)
    last_exc = None
    for attempt in range(3):
        try:
            res = run_bass_kernel_spmd(nc, in_maps, core_ids=list(range(NCORES)),
                                       trace=False)
            break
        except Exception as e:  # transient NRT/axon wedges recover on retry
            last_exc = e
            _time.sleep(5.0 * (attempt + 1))
    else:
        raise last_exc
    out = np.concatenate([res.results[m]["out"] for m in range(NCORES)], axis=0)
    return np.ascontiguousarray(out, dtype=np.float32)
